# revision 40
# baseline (speedup 1.0000x reference)
"""GQA causal attention (RoPE, B=2 S=2048 D=2048 H=16 KV=8 HD=128) on 8 trn2 cores.

Strategy: head-parallel. Each core c owns q-heads {2c, 2c+1} and kv-head c.
Host replicates x (pre-transposed to [D, B*S], bf16) to all cores; all
projections, RoPE and causal attention are head-sharded (zero comm). Two
AllToAlls (one per local q-head, 1 MB/rank each, bf16) convert the attention
output from head-sharding to row-sharding overlapped with the other head's
attention, then each core computes its 512-row slice of the output projection
with the full Wo. Host concatenates the 8 row shards.

Layout trick: everything is computed transposed (qT/kT = [HD, seq] with HD on
partitions, scores as [k, q]) so no on-device activation transposes are
needed; vT -> v runs on the DMA-transpose crossbar. Softmax runs max-free
(scores are small by construction), the denominator comes from four
column-tiled ones-vector matmul chains running concurrently on the PE, and
the causal mask is a 0/1 multiply on the vector engine over the 128-wide
diagonal window. All matmuls run bf16 (1 cyc/row on the PE; fp32 accumulates
in PSUM).

DMA rings: the sync HWDGE ring carries the latency-critical x-chunk feed plus
a2a stores and the output; the scalar HWDGE ring carries weights, rope
tables, the (chunk-spread) Wo prefetch, v transposes and a2a landing, so
bulk prefetches never head-of-line-block the x feed.
"""

import os
import sys

import numpy as np

if "/opt/trn_rl_repo" not in sys.path:
    sys.path.insert(0, "/opt/trn_rl_repo")

CORES = 8


def build_nc(B, S, D, H, KV, HD, HO, QC):
    """Build the SPMD bass graph (same graph for all 8 cores)."""
    import concourse.bacc as bacc
    import concourse.tile as tile
    from concourse import mybir
    from contextlib import ExitStack

    f32 = mybir.dt.float32
    bf16 = mybir.dt.bfloat16
    ACT = mybir.ActivationFunctionType
    ALU = mybir.AluOpType

    QH = H // CORES               # q heads per core (2)
    R = B * S                     # total rows (4096)
    RO = R // CORES               # output rows per core (512) == QC
    assert QC == RO
    DK = D // 128                 # k-tiles over model dim (16)
    RC = 512                      # row-chunk width for projections
    NCH = R // RC                 # projection row chunks (8)
    NQC = S // QC                 # q chunks per batch (4)
    NKT = S // 128                # k tiles per batch (16)
    NT = QC // 128                # diagonal mask patterns (4)
    NRT = RO // 128               # out row tiles per core (4)
    OC = min(D, 512)              # out col chunk
    NOC = D // OC                 # out col chunks (4)
    HG = H                        # total heads in O-proj
    scale = float(HD) ** -0.5

    nc = bacc.Bacc("TRN2", target_bir_lowering=False, debug=False,
                   num_devices=CORES)

    xT = nc.dram_tensor("xT", [D, R], bf16, kind="ExternalInput")
    cosT = nc.dram_tensor("cosT", [HD, S], bf16, kind="ExternalInput")
    sinT = nc.dram_tensor("sinT", [HD, S], bf16, kind="ExternalInput")
    wq = nc.dram_tensor("wq", [D, QH * HD], bf16, kind="ExternalInput")
    wk = nc.dram_tensor("wk", [D, HD], bf16, kind="ExternalInput")
    wv = nc.dram_tensor("wv", [D, HD], bf16, kind="ExternalInput")
    wo = nc.dram_tensor("wo", [HO, D], bf16, kind="ExternalInput")
    mask01 = nc.dram_tensor("mask01", [128, 128], bf16, kind="ExternalInput")
    ones = nc.dram_tensor("ones", [128, 128], bf16, kind="ExternalInput")
    out = nc.dram_tensor("out", [RO, D], f32, kind="ExternalOutput")

    with tile.TileContext(nc) as tc, ExitStack() as top:
        dram = top.enter_context(tc.tile_pool(name="dram", bufs=1, space="DRAM"))
        consts = top.enter_context(tc.tile_pool(name="consts", bufs=1))
        resid = top.enter_context(tc.tile_pool(name="resid", bufs=1))

        a2a_in = [dram.tile([CORES, 128, QC], bf16, name=f"a2ain{h}")
                  for h in range(QH)]
        a2a_out = [dram.tile([CORES, 128, QC], bf16, name=f"a2aout{h}")
                   for h in range(QH)]
        warm_in = dram.tile([CORES, 8], bf16, name="warm_in")
        warm_out = dram.tile([CORES, 8], bf16, name="warm_out")

        ones_sb = consts.tile([128, 128], bf16)
        mask01_sb = consts.tile([128, 128], bf16)

        # residents produced by projection phase, consumed by attention
        qT_sb = resid.tile([128, QH, R], bf16)       # [hd, head, row]
        kT_sb = resid.tile([128, R], bf16)           # [hd, row]
        v_sb = resid.tile([128, R], bf16)            # [kpos%128, ktile*HD+hd]

        # full Wo resident (prefetch spread over projection chunks; no deps)
        wo_all = resid.tile([128, HG, D], bf16)
        attn_all = resid.tile([128, HG, QC], bf16)

        # ------------------------------- phase 1: projections + rope
        with ExitStack() as ph1:
            ropec = ph1.enter_context(tc.tile_pool(name="ropec", bufs=1))
            wpool = ph1.enter_context(tc.tile_pool(name="wpool", bufs=1))
            xpool = ph1.enter_context(tc.tile_pool(name="xpool", bufs=7))
            rtmp = ph1.enter_context(tc.tile_pool(name="rtmp", bufs=3))
            psA = ph1.enter_context(tc.tile_pool(name="psA", bufs=6, space="PSUM"))

            wq_sb = wpool.tile([128, DK, QH * HD], bf16)
            wk_sb = wpool.tile([128, DK, HD], bf16)
            wv_sb = wpool.tile([128, DK, HD], bf16)
            cos_sb = ropec.tile([128, S], bf16)
            sin_sb = ropec.tile([128, S], bf16)
            wq_r = wq.ap().rearrange("(kt p) c -> p kt c", p=128)
            wk_r = wk.ap().rearrange("(kt p) c -> p kt c", p=128)
            wv_r = wv.ap().rearrange("(kt p) c -> p kt c", p=128)

            # chunk-0 x + weights interleaved kt-granular so MM kt can start
            # as soon as its own pieces land (subtile deps). x rides the sync
            # ring alone; weights/rope tables go on the scalar ring.
            DKH = DK // 2
            xch0 = [xpool.tile([128, DKH, RC], bf16, tag="xch", name="xch0")
                    for _ in range(2)]
            xsrc0 = xT[:, 0:RC].rearrange("(kt p) c -> p kt c", p=128)
            ncs = S // 512
            # weight issue order matches first-use order: the first chains
            # need only wq; wk/wv follow; rope tables after that
            for kt in range(DK):
                nc.sync.dma_start(out=xch0[kt // DKH][:, kt % DKH, :],
                                  in_=xsrc0[:, kt, :])
                nc.scalar.dma_start(out=wq_sb[:, kt, :], in_=wq_r[:, kt, :])
            for kt in range(DK):
                nc.scalar.dma_start(out=wk_sb[:, kt, :], in_=wk_r[:, kt, :])
                nc.scalar.dma_start(out=wv_sb[:, kt, :], in_=wv_r[:, kt, :])
                if kt % 4 == 3:
                    cs = kt // 4
                    if cs < ncs:
                        sl = slice(cs * 512, (cs + 1) * 512)
                        nc.scalar.dma_start(out=cos_sb[:, sl], in_=cosT[:, sl])
                        nc.scalar.dma_start(out=sin_sb[:, sl], in_=sinT[:, sl])
            for cs in range(DK // 4, ncs):
                sl = slice(cs * 512, (cs + 1) * 512)
                nc.scalar.dma_start(out=cos_sb[:, sl], in_=cosT[:, sl])
                nc.scalar.dma_start(out=sin_sb[:, sl], in_=sinT[:, sl])
            nc.scalar.dma_start(out=ones_sb, in_=ones[:, :])
            nc.scalar.dma_start(out=mask01_sb, in_=mask01[:, :])
            # tiny warm-up collective: absorbs the ~11us first-collective
            # spin-up so the real AllToAlls start promptly
            warm_sb = consts.tile([1, CORES * 8], bf16)
            nc.gpsimd.memset(warm_sb, 0)
            nc.sync.dma_start(out=warm_in.rearrange("g c -> (g c)"),
                              in_=warm_sb[0])
            from concourse import mybir as _mbw
            nc.gpsimd.collective_compute(
                "AllToAll", _mbw.AluOpType.bypass,
                ins=[warm_in.opt()], outs=[warm_out.opt()],
                replica_groups=[list(range(CORES))])
            # eager chunk-1 prefetch keeps the PE from idling (and HAM from
            # re-throttling) while the DMA rings ramp up
            xch1 = None
            if NCH > 1:
                xch1 = [xpool.tile([128, DKH, RC], bf16, tag="xch", name="xch1")
                        for _ in range(2)]
                xsrc1 = xT[:, RC:2 * RC].rearrange("(kt p) c -> p kt c", p=128)
                for hh in range(2):
                    for q2 in range(2):
                        sl = slice(q2 * DKH // 2, (q2 + 1) * DKH // 2)
                        nc.sync.dma_start(
                            out=xch1[hh][:, sl, :],
                            in_=xsrc1[:, hh * DKH + q2 * DKH // 2:
                                      hh * DKH + (q2 + 1) * DKH // 2, :])

            half = HD // 2

            def rope(pp, dst, poff):
                c_sl = cos_sb[:, poff:poff + RC]
                s_sl = sin_sb[:, poff:poff + RC]
                t1 = rtmp.tile([128, RC], f32, tag="t1", name="t1")
                t2 = rtmp.tile([128, RC], f32, tag="t2", name="t2")
                nc.vector.tensor_mul(t1, pp, c_sl)
                nc.vector.tensor_mul(t2[0:half, :], pp[half:128, :], s_sl[0:half, :])
                nc.vector.tensor_mul(t2[half:128, :], pp[0:half, :], s_sl[half:128, :])
                nc.vector.tensor_add(dst, t1, t2)

            # Wo prefetch: head-slices spread over chunks >= 2 on the scalar
            # ring (never bursting ahead of the x feed)
            wo_r = wo.ap().rearrange("(g p) n -> p g n", p=128)
            wo_sl = [slice(q8 * HG // 8, (q8 + 1) * HG // 8) for q8 in range(8)]
            start_ch = max(0, min(2, NCH - 1))
            nspread = NCH - start_ch
            wo_chunk = [NCH - 1 - ((7 - q8) % nspread) for q8 in range(8)]

            for n in range(NCH):
                if n == 0:
                    xs = xch0
                elif n == 1:
                    xs = xch1
                else:
                    xsrc = xT[:, n * RC:(n + 1) * RC].rearrange(
                        "(kt p) c -> p kt c", p=128)
                    xs = []
                    for hh in range(2):
                        xc = xpool.tile([128, DKH, RC], bf16, tag="xch", name="xch")
                        for q2 in range(2):
                            sl = slice(q2 * DKH // 2, (q2 + 1) * DKH // 2)
                            nc.sync.dma_start(
                                out=xc[:, sl, :],
                                in_=xsrc[:, hh * DKH + q2 * DKH // 2:
                                         hh * DKH + (q2 + 1) * DKH // 2, :])
                        xs.append(xc)
                for q8 in range(8):
                    if wo_chunk[q8] == n:
                        nc.scalar.dma_start(out=wo_all[:, wo_sl[q8], :],
                                            in_=wo_r[:, wo_sl[q8], :])

                poff = (n * RC) % S
                for oi in range(QH + 2):   # QH q heads, then k, then vT
                    pp = psA.tile([128, RC], f32, tag="pp", name="pp")
                    if oi < QH:
                        wcol = (wq_sb, oi * HD)
                    elif oi == QH:
                        wcol = (wk_sb, 0)
                    else:
                        wcol = (wv_sb, 0)
                    for kt in range(DK):
                        wsb = wcol[0][:, kt, wcol[1]:wcol[1] + HD]
                        nc.tensor.matmul(
                            pp, lhsT=wsb, rhs=xs[kt // DKH][:, kt % DKH, :],
                            start=(kt == 0), stop=(kt == DK - 1))
                    if oi < QH:
                        rope(pp, qT_sb[:, oi, n * RC:(n + 1) * RC], poff)
                    elif oi == QH:
                        rope(pp, kT_sb[:, n * RC:(n + 1) * RC], poff)
                    else:
                        vt_sb = rtmp.tile([128, RC], bf16, tag="vt", name="vt")
                        # copy on the DVE so the scalar queue stays clear
                        # for the attention exp stream
                        nc.vector.tensor_copy(vt_sb, pp)
                        # vT -> v on the DMA-transpose crossbar (off the PE):
                        # one instruction per chunk; transposed rows fold into
                        # (partition, block) pairs of the 3D out pattern
                        vdst = v_sb[:, n * RC:(n + 1) * RC].rearrange(
                            "p (j c) -> p j c", j=RC // 128)
                        nc.scalar.dma_start(out=vdst, in_=vt_sb,
                                            transpose=True)

        # ------------------------------- phase 2: attention (h-outer; the
        # first head's AllToAll overlaps the second head's attention)
        with ExitStack() as ph2:
            probs = ph2.enter_context(tc.tile_pool(name="probs", bufs=10))
            atmp = ph2.enter_context(tc.tile_pool(name="atmp", bufs=3))
            dens = ph2.enter_context(tc.tile_pool(name="dens", bufs=2))
            psS = ph2.enter_context(tc.tile_pool(name="psS", bufs=2, space="PSUM"))
            psO = ph2.enter_context(tc.tile_pool(name="psO", bufs=2, space="PSUM"))
            psX = ph2.enter_context(tc.tile_pool(name="psX", bufs=2, space="PSUM"))

            from concourse import mybir as _mb
            for h in range(QH):
                for b in range(B):
                    for qc in range(NQC - 1, -1, -1):
                        nkt = (qc + 1) * NT
                        po = psO.tile([128, QC], f32, tag="po", name="po")
                        # pden and the later broadcast share one bank: the
                        # broadcast overwrites it after the den rows are read
                        pden = psX.tile([128, QC], f32, tag="dn", name="pden")
                        prs = {}
                        offs = {}
                        # k-tiles are processed in pairs sharing a double-wide
                        # score tile so the exp runs as one activation per
                        # pair (the scalar exp stream paces this whole phase)
                        for t in range((nkt + 1) // 2):
                            kts = [kt for kt in (2 * t, 2 * t + 1) if kt < nkt]
                            sc2 = psS.tile([128, 2 * QC], f32, tag="sc", name="sc")
                            pr2 = probs.tile([128, 2 * QC], bf16, tag="pr", name="pr")
                            for m, kt in enumerate(kts):
                                dj = kt - qc * NT   # >=0 on diagonal block
                                o = max(dj, 0) * 128
                                offs[kt] = o
                                prs[kt] = (pr2, m * QC)
                                kl = kT_sb[:, b * S + kt * 128: b * S + (kt + 1) * 128]
                                nc.tensor.matmul(
                                    sc2[:, m * QC + o:(m + 1) * QC], lhsT=kl,
                                    rhs=qT_sb[:, h, b * S + qc * QC + o: b * S + (qc + 1) * QC],
                                    start=True, stop=True)
                            if len(kts) == 2 and offs[kts[0]] == 0 and offs[kts[1]] == 0:
                                nc.scalar.activation(pr2, sc2, ACT.Exp, scale=scale)
                            else:
                                for m, kt in enumerate(kts):
                                    o = offs[kt]
                                    nc.scalar.activation(
                                        pr2[:, m * QC + o:(m + 1) * QC],
                                        sc2[:, m * QC + o:(m + 1) * QC],
                                        ACT.Exp, scale=scale)
                            for m, kt in enumerate(kts):
                                if kt - qc * NT >= 0:
                                    # causal 0/1 mask over the 128-wide
                                    # diagonal window
                                    o = offs[kt]
                                    nc.vector.tensor_mul(
                                        pr2[:, m * QC + o:m * QC + o + 128],
                                        pr2[:, m * QC + o:m * QC + o + 128],
                                        mask01_sb)
                        for kt in range(nkt):
                            ktg = b * NKT + kt
                            o = offs[kt]
                            pt, pb = prs[kt]
                            nc.tensor.matmul(
                                po[:, o:QC], lhsT=v_sb[:, ktg * 128:(ktg + 1) * 128],
                                rhs=pt[:, pb + o:pb + QC],
                                start=(kt == 0), stop=(kt == nkt - 1))
                        # denominator: 4 column-tiled ones-vector chains run
                        # concurrently in distinct 32-col groups of the PE
                        nch = min(4, nkt)
                        for kt in range(nkt):
                            j = kt % 4
                            o = offs[kt]
                            pt, pb = prs[kt]
                            nc.tensor.matmul(
                                pden[32 * j:32 * j + 1, o:QC],
                                lhsT=ones_sb[:, 0:1],
                                rhs=pt[:, pb + o:pb + QC],
                                start=(kt < 4), stop=(kt >= nkt - 4),
                                tile_position=(0, 32 * j),
                                skip_group_check=True)
                        den = dens.tile([1, QC], f32, tag="den", name="den")
                        nc.vector.tensor_copy(den, pden[0:1, :])
                        for j in range(1, nch):
                            vj = offs[j]
                            nc.vector.tensor_add(
                                den[:, vj:QC], den[:, vj:QC],
                                pden[32 * j:32 * j + 1, vj:QC])
                        denr = dens.tile([1, QC], f32, tag="denr", name="denr")
                        nc.vector.reciprocal_approx_fast(denr, den)
                        denb = dens.tile([1, QC], bf16, tag="denb", name="denb")
                        nc.vector.tensor_copy(denb, denr)
                        # broadcast 1/den to 128 partitions via ones-column MM
                        # (bf16: fp32 matmul lowers to a slow two-pass mode)
                        nc.tensor.matmul(pden, lhsT=ones_sb[0:1, :], rhs=denb,
                                         start=True, stop=True)
                        pbc = pden
                        # normalize: po stays in PSUM (DVE may read one PSUM
                        # operand); 1/den broadcast goes via a quick SBUF hop
                        # on the DVE (scalar must stay a pure exp stream)
                        bc = atmp.tile([128, QC], bf16, tag="bc", name="bc")
                        nc.vector.tensor_copy(bc, pbc)
                        d = b * NQC + qc
                        anorm = atmp.tile([128, QC], bf16, tag="an", name="anorm")
                        nc.vector.scalar_tensor_tensor(
                            anorm, in0=po, scalar=1.0, in1=bc,
                            op0=ALU.bypass, op1=ALU.mult)
                        nc.sync.dma_start(out=a2a_in[h][d], in_=anorm)

                nc.gpsimd.collective_compute(
                    "AllToAll", _mb.AluOpType.bypass,
                    ins=[a2a_in[h].opt()], outs=[a2a_out[h].opt()],
                    replica_groups=[list(range(CORES))])

            # land both heads' shares of attn_all AFTER both collective
            # triggers are queued: nothing may sit ahead of the second
            # trigger in any FIFO, or its collective starts a2a0-late.
            # h0 lands via the scalar queue (free right after the last exp);
            # h1 via sync (right after its own a2a stores). Per-rank DMAs so
            # the o-proj chains resume in arrival order.
            for h in range(QH):
                asrc = a2a_out[h].rearrange("g p q -> p g q")
                adst = attn_all.rearrange("p (g hl) q -> p g hl q", hl=QH)
                eng = nc.sync if h == QH - 1 else nc.scalar
                ngrp = CORES if h == QH - 1 else 4
                for qg in range(ngrp):
                    sl = slice(qg * CORES // ngrp, (qg + 1) * CORES // ngrp)
                    eng.dma_start(out=adst[:, sl, h, :], in_=asrc[:, sl, :])

        # ------------------------------- phase 3: output projection in two
        # waves: even heads (first AllToAll) accumulate to SBUF while the
        # second AllToAll flies; odd heads then add on top.
        with ExitStack() as ph3:
            outp = ph3.enter_context(tc.tile_pool(name="outp", bufs=4))
            stage = ph3.enter_context(tc.tile_pool(name="stage", bufs=1))
            psP = ph3.enter_context(tc.tile_pool(name="psP", bufs=2, space="PSUM"))
            # even-head partial of the output projection (phase 3 staging)
            even_sb = stage.tile([128, NRT, D], f32)

            evens = [g for g in range(HG) if g % QH == 0]
            odds = [g for g in range(HG) if g % QH != 0]
            for rt in range(NRT):
                pp = [psP.tile([128, OC], f32, tag=f"ppo{oc}", name=f"ppo{oc}")
                      for oc in range(NOC)]
                for gi, g in enumerate(evens):
                    al = attn_all[:, g, rt * 128:(rt + 1) * 128]
                    for oc in range(NOC):
                        nc.tensor.matmul(
                            pp[oc], lhsT=al, rhs=wo_all[:, g, oc * OC:(oc + 1) * OC],
                            start=(gi == 0), stop=(gi == len(evens) - 1))
                for oc in range(NOC):
                    nc.scalar.activation(even_sb[:, rt, oc * OC:(oc + 1) * OC],
                                         pp[oc], ACT.Copy)
            for rt in range(NRT):
                pp = [psP.tile([128, OC], f32, tag=f"ppo{oc}", name=f"ppo{oc}")
                      for oc in range(NOC)]
                for gi, g in enumerate(odds):
                    al = attn_all[:, g, rt * 128:(rt + 1) * 128]
                    for oc in range(NOC):
                        nc.tensor.matmul(
                            pp[oc], lhsT=al, rhs=wo_all[:, g, oc * OC:(oc + 1) * OC],
                            start=(gi == 0), stop=(gi == len(odds) - 1))
                for oc in range(NOC):
                    osb = outp.tile([128, OC], f32, tag="osb", name="osb")
                    nc.vector.tensor_add(osb, pp[oc],
                                         even_sb[:, rt, oc * OC:(oc + 1) * OC])
                    nc.sync.dma_start(out=out[rt * 128:(rt + 1) * 128, oc * OC:(oc + 1) * OC],
                                      in_=osb)

    nc.compile()
    return nc


def make_in_maps(x, cos, sin, Wq, Wk, Wv, Wo, QC):
    import ml_dtypes
    bf = ml_dtypes.bfloat16
    B, S, D = x.shape
    HD = cos.shape[1]
    H = Wq.shape[1] // HD
    QH = H // CORES
    R = B * S

    xT = np.ascontiguousarray(x.reshape(R, D).T).astype(bf)
    cosT = np.ascontiguousarray(cos.T).astype(bf)
    sT = sin.T.astype(np.float32)
    half = HD // 2
    sinTs = np.ascontiguousarray(
        np.concatenate([-sT[:half], sT[half:]], axis=0)).astype(bf)

    kk = np.arange(128)[:, None]
    qq = np.arange(128)[None, :]
    mask01 = (qq >= kk).astype(np.float32).astype(bf)

    in_maps = []
    for c in range(CORES):
        in_maps.append({
            "xT": xT,
            "cosT": cosT,
            "sinT": sinTs,
            "wq": np.ascontiguousarray(Wq[:, c * QH * HD:(c + 1) * QH * HD]).astype(bf),
            "wk": np.ascontiguousarray(Wk[:, c * HD:(c + 1) * HD]).astype(bf),
            "wv": np.ascontiguousarray(Wv[:, c * HD:(c + 1) * HD]).astype(bf),
            "wo": np.asarray(Wo).astype(bf),
            "mask01": mask01,
            "ones": np.ones((128, 128), dtype=bf),
        })
    return in_maps


def _install_profile_shim():
    """Provide antenv.axon_hooks (missing in this image) so
    run_bass_kernel_spmd(trace=True) can capture NTFF profiles via the
    axon PJRT .so; also neuter the artifact upload."""
    import types

    try:
        import antenv.axon_hooks  # noqa: F401
    except ImportError:
        from trn_agent_boot.trn_boot import _ntff_profile_via_ctypes
        hook = _ntff_profile_via_ctypes("/opt/axon/libaxon_pjrt.so")
        if hook is None:
            raise RuntimeError("libaxon_pjrt.so lacks profile symbols")
        mod = types.ModuleType("antenv.axon_hooks")
        mod.get_axon_ntff_profile_hook = lambda: hook
        mod.set_axon_ntff_profile_hook = lambda h: None
        sys.modules["antenv.axon_hooks"] = mod
        import antenv
        antenv.axon_hooks = mod
    import concourse.bass_utils as bu
    bu.upload_artifacts = lambda tmpdir: str(tmpdir)


_NC_CACHE = {}


def _get_nc(B, S, D, H, KV, HD, HO, QC):
    key = (B, S, D, H, KV, HD, HO, QC)
    if key not in _NC_CACHE:
        _NC_CACHE[key] = build_nc(B, S, D, H, KV, HD, HO, QC)
    return _NC_CACHE[key]


def kernel(x, cos, sin, Wq, Wk, Wv, Wo, _sim=False):
    x = np.asarray(x, dtype=np.float32)
    cos = np.asarray(cos, dtype=np.float32)
    sin = np.asarray(sin, dtype=np.float32)
    Wq = np.asarray(Wq, dtype=np.float32)
    Wk = np.asarray(Wk, dtype=np.float32)
    Wv = np.asarray(Wv, dtype=np.float32)
    Wo = np.asarray(Wo, dtype=np.float32)

    B, S, D = x.shape
    HD = cos.shape[1]
    H = Wq.shape[1] // HD
    KV = Wk.shape[1] // HD
    HO = Wq.shape[1]
    R = B * S
    QC = R // CORES

    nc = _get_nc(B, S, D, H, KV, HD, HO, QC)
    in_maps = make_in_maps(x, cos, sin, Wq, Wk, Wv, Wo, QC)

    if _sim:
        from concourse import bass_interp
        sim = bass_interp.MultiCoreSim(nc, CORES)
        for c in range(CORES):
            for k, v in in_maps[c].items():
                sim.cores[c].tensor(k)[:] = v
        sim.simulate(check_with_hw=False)
        shards = [np.array(sim.cores[c].mem_tensor("out")) for c in range(CORES)]
    else:
        from concourse.bass_utils import run_bass_kernel_spmd
        trace = os.environ.get("KERNEL_TRACE", "0") == "1"
        res = None
        if trace:
            try:
                _install_profile_shim()
                tmpdir = os.environ.get("KERNEL_TMPDIR") or None
                res = run_bass_kernel_spmd(nc, in_maps,
                                           core_ids=list(range(CORES)),
                                           trace=True, tmpdir=tmpdir)
            except Exception as e:  # fall back to untraced run
                print(f"traced run failed ({type(e).__name__}: {e}); "
                      f"retrying untraced")
                res = None
        if res is None:
            res = run_bass_kernel_spmd(nc, in_maps,
                                       core_ids=list(range(CORES)),
                                       trace=False)
        if res.exec_time_ns is not None:
            print(f"HW exec time: {res.exec_time_ns} ns")
        shards = [res.results[c]["out"] for c in range(CORES)]

    return np.concatenate(shards, axis=0).reshape(B, S, D).astype(np.float32)
